# revision 3
# baseline (speedup 1.0000x reference)
"""BRU (bistable recurrent unit) cell kernel for 8 Trainium2 NeuronCores.

Hardcoded problem: B=64, T=512, D=1024, U=1024, fp32.

Sharding: 8 cores = 4 batch-groups (16 batches each) x 2 unit-groups
(512 units each).  Per core the three input projections
    projT[u, token] = K[d,u].T @ xT[d, token],   token = b*512 + t
run on the PE in fp16 with a 3-term split for fp32-grade accuracy:
    x @ K  ~=  A@K1 + e@K1 + (A*2^-12)@(K2*2^12)
with A = fp16(x), e = fp16(x - A), K1 = fp16(K), K2 = K - K1 (scaled by
2^12 into fp16 normal range; the power-of-two scales cancel exactly).
All terms run at 1 cycle/row on the PE and accumulate in fp32 PSUM.

The 512-step recurrence is elementwise with u on partitions:
state [128, (uh=4, b=16)].  Per step: 7 DVE ops + 3 ACT ops using
scalar_tensor_tensor fusions and h' = z'*hh + (1-z')*h with
z' = sigmoid(-zin) = 1-z.  Projections are chunked TC steps at a time
(double-buffered) so the PE runs ahead of the recurrence; PSUM->SBUF
copies ride on the Scalar engine and fold in the bias when nonzero.
"""

import os

import numpy as np

B, T, D, U = 64, 512, 1024, 1024
NCORES = 8
NBG = 4  # batch groups
NUG = 2  # unit groups
BL = B // NBG  # 16 batches per core
UHALF = U // NUG  # 512 units per core
UH = UHALF // 128  # 4 u-chunks

_CACHE: dict = {}


def _build(T_, TC, use_memory, use_bias):
    """Build and compile the per-core Bass program."""
    import concourse.mybir as mybir
    from concourse import bacc
    from concourse.tile import TileContext

    f32 = mybir.dt.float32
    f16 = mybir.dt.float16
    Alu = mybir.AluOpType
    Act = mybir.ActivationFunctionType

    NTOK = BL * T_
    NCH = T_ // TC
    DC = D // 128  # 8 d-chunks

    nc = bacc.Bacc("TRN2", target_bir_lowering=False, debug=False)

    xA = nc.dram_tensor("xA", [D, NTOK], f16, kind="ExternalInput").ap()
    xE = nc.dram_tensor("xE", [D, NTOK], f16, kind="ExternalInput").ap()
    xS = nc.dram_tensor("xS", [D, NTOK], f16, kind="ExternalInput").ap()
    k1 = {}
    k2 = {}
    for g in "zrh":
        k1[g] = nc.dram_tensor(f"k1{g}", [D, UHALF], f16, kind="ExternalInput").ap()
        k2[g] = nc.dram_tensor(f"k2{g}", [D, UHALF], f16, kind="ExternalInput").ap()
    if use_memory:
        mzb = nc.dram_tensor("mzb", [128, UH, BL], f32, kind="ExternalInput").ap()
        mrb = nc.dram_tensor("mrb", [128, UH, BL], f32, kind="ExternalInput").ap()
    if use_bias:
        bts = {
            g: nc.dram_tensor(f"bt{g}", [128, UH], f32, kind="ExternalInput").ap()
            for g in "zrh"
        }
    outT = nc.dram_tensor("outT", [UHALF, NTOK], f32, kind="ExternalOutput").ap()

    xA_r = xA.rearrange("(dc p) (b t) -> dc p b t", dc=DC, b=BL)
    xE_r = xE.rearrange("(dc p) (b t) -> dc p b t", dc=DC, b=BL)
    xS_r = xS.rearrange("(dc p) (b t) -> dc p b t", dc=DC, b=BL)
    outT_r = outT.rearrange("(uh p) (b t) -> uh p b t", uh=UH, b=BL)

    with TileContext(nc) as tc:
        with (
            tc.tile_pool(name="weights", bufs=1) as wpool,
            tc.tile_pool(name="xin", bufs=2) as xpool,
            tc.tile_pool(name="proj", bufs=2) as ppool,
            tc.tile_pool(name="hout", bufs=2) as hpool,
            tc.tile_pool(name="tmp", bufs=3) as spool,
            tc.tile_pool(name="misc", bufs=1) as mpool,
            tc.tile_pool(name="psum", bufs=4, space="PSUM") as qpool,
        ):
            w1 = {}
            w2 = {}
            for g in "zrh":
                w1[g] = wpool.tile([128, DC, UHALF], f16, tag=f"w1{g}", name=f"w1{g}")
                w2[g] = wpool.tile([128, DC, UHALF], f16, tag=f"w2{g}", name=f"w2{g}")
                nc.sync.dma_start(
                    w1[g][:, :, :], k1[g].rearrange("(dc p) u -> p dc u", p=128)
                )
                nc.sync.dma_start(
                    w2[g][:, :, :], k2[g].rearrange("(dc p) u -> p dc u", p=128)
                )
            if use_memory:
                mz_t = mpool.tile([128, UH, BL], f32, tag="mz")
                mr_t = mpool.tile([128, UH, BL], f32, tag="mr")
                nc.sync.dma_start(mz_t[:, :, :], mzb[:, :, :])
                nc.sync.dma_start(mr_t[:, :, :], mrb[:, :, :])
            if use_bias:
                b_t = {}
                for g in "zrh":
                    b_t[g] = mpool.tile([128, UH], f32, tag=f"b{g}", name=f"b{g}")
                    nc.sync.dma_start(b_t[g][:, :], bts[g][:, :])

            h0 = mpool.tile([128, UH, BL], f32, tag="h0")
            nc.gpsimd.memset(h0[:, :, :], 0.0)

            prev_h = h0
            for c in range(NCH):
                t0 = c * TC
                xa = xpool.tile([128, DC, BL, TC], f16, tag="xa")
                xe = xpool.tile([128, DC, BL, TC], f16, tag="xe")
                xs = xpool.tile([128, DC, BL, TC], f16, tag="xs")
                for dc in range(DC):
                    nc.sync.dma_start(xa[:, dc, :, :], xA_r[dc, :, :, t0 : t0 + TC])
                    nc.sync.dma_start(xe[:, dc, :, :], xE_r[dc, :, :, t0 : t0 + TC])
                    nc.sync.dma_start(xs[:, dc, :, :], xS_r[dc, :, :, t0 : t0 + TC])
                projs = {}
                for g in "zrh":
                    pg = ppool.tile([128, UH, BL, TC], f32, tag=f"p{g}", name=f"p{g}_{c}")
                    projs[g] = pg
                    for uh in range(UH):
                        us = slice(uh * 128, (uh + 1) * 128)
                        ps = qpool.tile([128, BL, TC], f32, tag="ps")
                        for dc in range(DC):
                            nc.tensor.matmul(
                                ps[:, :, :], w1[g][:, dc, us], xa[:, dc, :, :],
                                start=(dc == 0), stop=False,
                            )
                            nc.tensor.matmul(
                                ps[:, :, :], w1[g][:, dc, us], xe[:, dc, :, :],
                                start=False, stop=False,
                            )
                        for dc in range(DC):
                            nc.tensor.matmul(
                                ps[:, :, :], w2[g][:, dc, us], xs[:, dc, :, :],
                                start=False, stop=(dc == DC - 1),
                            )
                        if use_bias:
                            nc.scalar.activation(
                                pg[:, uh, :, :], ps[:, :, :], Act.Identity,
                                bias=b_t[g][:, uh : uh + 1],
                            )
                        else:
                            nc.scalar.activation(
                                pg[:, uh, :, :], ps[:, :, :], Act.Identity,
                            )

                pz, pr, ph = projs["z"], projs["r"], projs["h"]
                hch = hpool.tile([128, UH, BL, TC], f32, tag="hch")
                for trel in range(TC):
                    h = (
                        prev_h[:, :, :]
                        if trel == 0 and c == 0
                        else (
                            prev_h[:, :, :, TC - 1]
                            if trel == 0
                            else hch[:, :, :, trel - 1]
                        )
                    )
                    xz_t = pz[:, :, :, trel]
                    xr_t = pr[:, :, :, trel]
                    xh_t = ph[:, :, :, trel]

                    t1in = spool.tile([128, UH, BL], f32, tag="t1in")
                    zin = spool.tile([128, UH, BL], f32, tag="zin")
                    if use_memory:
                        hm_r = spool.tile([128, UH, BL], f32, tag="hm_r")
                        hm_z = spool.tile([128, UH, BL], f32, tag="hm_z")
                        nc.vector.tensor_mul(hm_r[:, :, :], h, mr_t[:, :, :])
                        nc.vector.tensor_add(t1in[:, :, :], hm_r[:, :, :], xr_t)
                        nc.vector.tensor_mul(hm_z[:, :, :], h, mz_t[:, :, :])
                        nc.vector.tensor_add(zin[:, :, :], hm_z[:, :, :], xz_t)
                    else:
                        nc.vector.tensor_add(t1in[:, :, :], h, xr_t)
                        nc.vector.tensor_add(zin[:, :, :], h, xz_t)

                    t1 = spool.tile([128, UH, BL], f32, tag="t1")
                    zp = spool.tile([128, UH, BL], f32, tag="zp")
                    nc.scalar.activation(t1[:, :, :], t1in[:, :, :], Act.Tanh)
                    # z' = sigmoid(-zin) = 1 - z
                    nc.scalar.activation(
                        zp[:, :, :], zin[:, :, :], Act.Sigmoid, scale=-1.0
                    )

                    # w2 = (t1 + 1) * h = r * h
                    rh = spool.tile([128, UH, BL], f32, tag="rh")
                    nc.vector.scalar_tensor_tensor(
                        rh[:, :, :], t1[:, :, :], 1.0, h, Alu.add, Alu.mult
                    )
                    hhin = spool.tile([128, UH, BL], f32, tag="hhin")
                    nc.vector.tensor_add(hhin[:, :, :], rh[:, :, :], xh_t)
                    hh = spool.tile([128, UH, BL], f32, tag="hh")
                    nc.scalar.activation(hh[:, :, :], hhin[:, :, :], Act.Tanh)

                    # h' = z'*hh + (1-z')*h  ==  e2 - a with
                    # e2 = z'*hh, a = (z'-1)*h
                    a = spool.tile([128, UH, BL], f32, tag="a")
                    nc.vector.scalar_tensor_tensor(
                        a[:, :, :], zp[:, :, :], 1.0, h, Alu.subtract, Alu.mult
                    )
                    e2 = spool.tile([128, UH, BL], f32, tag="e2")
                    nc.vector.tensor_mul(e2[:, :, :], zp[:, :, :], hh[:, :, :])
                    nc.vector.tensor_sub(hch[:, :, :, trel], e2[:, :, :], a[:, :, :])

                for uh in range(UH):
                    nc.sync.dma_start(
                        outT_r[uh, :, :, t0 : t0 + TC], hch[:, uh, :, :]
                    )
                prev_h = hch

    nc.compile()
    return nc


def _get_nc(T_, TC, use_memory, use_bias):
    key = (T_, TC, use_memory, use_bias)
    if key not in _CACHE:
        _CACHE[key] = _build(T_, TC, use_memory, use_bias)
    return _CACHE[key]


def kernel(
    x,
    kernel_z,
    kernel_r,
    kernel_h,
    memory_z,
    memory_r,
    bias_z,
    bias_r,
    bias_h,
):
    from concourse import bass_utils

    x = np.asarray(x, dtype=np.float32)
    Ks = {
        "z": np.asarray(kernel_z, dtype=np.float32),
        "r": np.asarray(kernel_r, dtype=np.float32),
        "h": np.asarray(kernel_h, dtype=np.float32),
    }
    mem = {
        "z": np.asarray(memory_z, dtype=np.float32),
        "r": np.asarray(memory_r, dtype=np.float32),
    }
    bias = {
        "z": np.asarray(bias_z, dtype=np.float32),
        "r": np.asarray(bias_r, dtype=np.float32),
        "h": np.asarray(bias_h, dtype=np.float32),
    }

    B_, T_, D_ = x.shape
    assert (B_, D_) == (B, D), (x.shape,)
    TC = int(os.environ.get("BRU_TC", "32"))

    use_memory = not all(np.all(m == 1.0) for m in mem.values())
    use_bias = not all(np.all(b == 0.0) for b in bias.values())

    nc = _get_nc(T_, TC, use_memory, use_bias)

    # Split weights once (shared across cores).
    w1_full = {}
    w2_full = {}
    for g, K in Ks.items():
        K1 = K.astype(np.float16)
        K2s = ((K - K1.astype(np.float32)) * 4096.0).astype(np.float16)
        w1_full[g] = K1
        w2_full[g] = K2s

    in_maps = []
    for c in range(NCORES):
        bg, ug = divmod(c, NUG)
        xc = x[bg * BL : (bg + 1) * BL].reshape(BL * T_, D)
        xcT = np.ascontiguousarray(xc.T)  # [D, NTOK] fp32
        A = xcT.astype(np.float16)
        e = (xcT - A.astype(np.float32)).astype(np.float16)
        As = (A.astype(np.float32) * (2.0 ** -12)).astype(np.float16)
        us = slice(ug * UHALF, (ug + 1) * UHALF)
        m = {"xA": A, "xE": e, "xS": As}
        for g in "zrh":
            m[f"k1{g}"] = np.ascontiguousarray(w1_full[g][:, us])
            m[f"k2{g}"] = np.ascontiguousarray(w2_full[g][:, us])
        if use_memory:
            # element (p, uh, b) = mem[ug*UHALF + uh*128 + p]
            for name, v in (("mzb", mem["z"]), ("mrb", mem["r"])):
                mv = v[us].reshape(UH, 128).T  # [128, UH]
                m[name] = np.ascontiguousarray(
                    np.broadcast_to(mv[:, :, None], (128, UH, BL))
                )
        if use_bias:
            for g in "zrh":
                m[f"bt{g}"] = np.ascontiguousarray(bias[g][us].reshape(UH, 128).T)
        in_maps.append(m)

    res = bass_utils.run_bass_kernel_spmd(nc, in_maps, core_ids=list(range(NCORES)))

    out = np.empty((B, T_, U), dtype=np.float32)
    for c in range(NCORES):
        bg, ug = divmod(c, NUG)
        oT = res.results[c]["outT"]  # [UHALF, BL*T_]
        out[bg * BL : (bg + 1) * BL, :, ug * UHALF : (ug + 1) * UHALF] = (
            oT.reshape(UHALF, BL, T_).transpose(1, 2, 0)
        )
    return out


# revision 11
# speedup vs baseline: 47.6033x; 47.6033x over previous
"""BRU (bistable recurrent unit) cell kernel for 8 Trainium2 NeuronCores.

Hardcoded problem: B=64, T=512, D=1024, U=1024, fp32.

Sharding: 8 cores = 4 batch-groups (16 batches each) x 2 unit-groups
(512 units each).  Per core the three input projections
    projT[u, token] = K[d,u].T @ xT[d, token],   token = b*512 + t
run on the PE in fp16 with a 3-term split for fp32-grade accuracy:
    x @ K  ~=  A@K1 + e@K1 + (A*2^-12)@(K2*2^12)
with A = fp16(x), e = fp16(x - A), K1 = fp16(K), K2 = K - K1 (scaled by
2^12 into fp16 normal range; the power-of-two scales cancel exactly).
All terms run at 1 cycle/row on the PE and accumulate in fp32 PSUM.

The 512-step recurrence is elementwise with u on partitions:
state [128, (uh=4, b=16)].  Per step: 7 DVE ops + 3 ACT ops using
scalar_tensor_tensor fusions and h' = z'*hh + (1-z')*h with
z' = sigmoid(-zin) = 1-z.  Projections are chunked TC steps at a time
(double-buffered) so the PE runs ahead of the recurrence; PSUM->SBUF
copies ride on the Scalar engine and fold in the bias when nonzero.
"""

import os

import numpy as np

B, T, D, U = 64, 512, 1024, 1024
NCORES = 8
NBG = 4  # batch groups
NUG = 2  # unit groups
BL = B // NBG  # 16 batches per core
UHALF = U // NUG  # 512 units per core
UH = UHALF // 128  # 4 u-chunks

_CACHE: dict = {}


def _build(T_, TC, use_memory, use_bias):
    """Build and compile the per-core Bass program."""
    import concourse.mybir as mybir
    from concourse import bacc
    from concourse.tile import TileContext

    f32 = mybir.dt.float32
    f16 = mybir.dt.float16
    Alu = mybir.AluOpType
    Act = mybir.ActivationFunctionType

    NTOK = BL * T_
    NCH = T_ // TC
    DC = D // 128  # 8 d-chunks

    nc = bacc.Bacc("TRN2", target_bir_lowering=False, debug=False)

    xA = nc.dram_tensor("xA", [D, NTOK], f16, kind="ExternalInput").ap()
    xE = nc.dram_tensor("xE", [D, NTOK], f16, kind="ExternalInput").ap()
    xS = nc.dram_tensor("xS", [D, NTOK], f16, kind="ExternalInput").ap()
    k1 = {}
    k2 = {}
    for g in "zrh":
        k1[g] = nc.dram_tensor(f"k1{g}", [D, UHALF], f16, kind="ExternalInput").ap()
        k2[g] = nc.dram_tensor(f"k2{g}", [D, UHALF], f16, kind="ExternalInput").ap()
    if use_memory:
        mzb = nc.dram_tensor("mzb", [128, UH, BL], f32, kind="ExternalInput").ap()
        mrb = nc.dram_tensor("mrb", [128, UH, BL], f32, kind="ExternalInput").ap()
    if use_bias:
        bts = {
            g: nc.dram_tensor(f"bt{g}", [128, UH], f32, kind="ExternalInput").ap()
            for g in "zrh"
        }
    outT = nc.dram_tensor("outT", [UHALF, NTOK], f32, kind="ExternalOutput").ap()

    xA_r = xA.rearrange("(dc p) (b t) -> dc p b t", dc=DC, b=BL)
    xE_r = xE.rearrange("(dc p) (b t) -> dc p b t", dc=DC, b=BL)
    xS_r = xS.rearrange("(dc p) (b t) -> dc p b t", dc=DC, b=BL)
    outT_r = outT.rearrange("(uh p) (b t) -> uh p b t", uh=UH, b=BL)

    # Tapered chunk schedule: full TC chunks, then short final chunks so the
    # exposed scan tail after the last matmul is only a few steps long.
    chunks = []
    rem = T_
    taper = [TC // 2, TC // 4, TC // 4] if T_ > 2 * TC else []
    full = (T_ - sum(taper)) // TC
    chunks = [TC] * full + taper
    assert sum(chunks) == T_, (chunks, T_)

    with TileContext(nc) as tc:
        with (
            tc.tile_pool(name="weights", bufs=1) as wpool,
            tc.tile_pool(name="xin", bufs=2) as xpool,
            tc.tile_pool(name="proj", bufs=2) as ppool,
            tc.tile_pool(name="hout", bufs=3) as hpool,
            tc.tile_pool(name="tmp", bufs=12) as spool,
            tc.tile_pool(name="misc", bufs=1) as mpool,
            tc.tile_pool(name="psum", bufs=8, space="PSUM") as qpool,
        ):
            # Startup order: z-gate weights, then the first x chunk, then
            # the remaining weights, so the PE's first PSUM group can start
            # as early as possible.
            TC0 = chunks[0]
            w1 = {}
            w2 = {}
            for g in "zrh":
                w1[g] = wpool.tile([128, DC, UHALF], f16, tag=f"w1{g}", name=f"w1{g}")
                w2[g] = wpool.tile([128, DC, UHALF], f16, tag=f"w2{g}", name=f"w2{g}")
            nc.sync.dma_start(
                w1["z"][:, :, :], k1["z"].rearrange("(dc p) u -> p dc u", p=128)
            )
            xa = xpool.tile([128, DC, BL, TC], f16, tag="xa", name="xa_0")
            xe = xpool.tile([128, DC, BL, TC], f16, tag="xe", name="xe_0")
            xs = xpool.tile([128, DC, BL, TC], f16, tag="xs", name="xs_0")
            for dc in range(DC):
                nc.sync.dma_start(xa[:, dc, :, :TC0], xA_r[dc, :, :, 0:TC0])
                nc.sync.dma_start(xe[:, dc, :, :TC0], xE_r[dc, :, :, 0:TC0])
            nc.sync.dma_start(
                w2["z"][:, :, :], k2["z"].rearrange("(dc p) u -> p dc u", p=128)
            )
            for dc in range(DC):
                nc.sync.dma_start(xs[:, dc, :, :TC0], xS_r[dc, :, :, 0:TC0])
            first_x = (xa, xe, xs)
            for g in "rh":
                nc.sync.dma_start(
                    w1[g][:, :, :], k1[g].rearrange("(dc p) u -> p dc u", p=128)
                )
                nc.sync.dma_start(
                    w2[g][:, :, :], k2[g].rearrange("(dc p) u -> p dc u", p=128)
                )
            if use_memory:
                # host passes mzb = 0.25*m_z, mrb = 0.5*m_r broadcasts
                mz4_t = mpool.tile([128, UH, BL], f32, tag="mz4", name="mz4")
                mr2_t = mpool.tile([128, UH, BL], f32, tag="mr2", name="mr2")
                nc.sync.dma_start(mz4_t[:, :, :], mzb[:, :, :])
                nc.sync.dma_start(mr2_t[:, :, :], mrb[:, :, :])
            if use_bias:
                b_t = {}
                for g in "zrh":
                    b_t[g] = mpool.tile([128, UH], f32, tag=f"b{g}", name=f"b{g}")
                    nc.sync.dma_start(b_t[g][:, :], bts[g][:, :])

            h0 = []
            for gi in range(2):
                h0g = mpool.tile([128, UH, BL // 2], f32, tag=f"h0{gi}", name=f"h0{gi}")
                nc.gpsimd.memset(h0g[:, :, :], 0.0)
                h0.append(h0g)

            # batch halves for the two-group pipelined scan
            GROUPS = ((0, BL // 2), (BL // 2, BL))

            prev_h = h0
            prev_tc = None
            t0 = 0
            for c, TCc in enumerate(chunks):
                if c == 0:
                    xa, xe, xs = first_x
                else:
                    xa = xpool.tile([128, DC, BL, TC], f16, tag="xa", name=f"xa_{c}")
                    xe = xpool.tile([128, DC, BL, TC], f16, tag="xe", name=f"xe_{c}")
                    xs = xpool.tile([128, DC, BL, TC], f16, tag="xs", name=f"xs_{c}")
                    for dc in range(DC):
                        nc.sync.dma_start(
                            xa[:, dc, :, :TCc], xA_r[dc, :, :, t0 : t0 + TCc]
                        )
                        nc.sync.dma_start(
                            xe[:, dc, :, :TCc], xE_r[dc, :, :, t0 : t0 + TCc]
                        )
                        nc.sync.dma_start(
                            xs[:, dc, :, :TCc], xS_r[dc, :, :, t0 : t0 + TCc]
                        )
                projs = {}
                for g in "zrh":
                    pg = ppool.tile(
                        [128, UH, BL, TC], f32, tag=f"p{g}", name=f"p{g}_{c}"
                    )
                    projs[g] = pg
                    for uh in range(UH):
                        us = slice(uh * 128, (uh + 1) * 128)
                        ps = qpool.tile([128, BL, TC], f32, tag="ps")
                        for dc in range(DC):
                            nc.tensor.matmul(
                                ps[:, :, :TCc], w1[g][:, dc, us], xa[:, dc, :, :TCc],
                                start=(dc == 0), stop=False,
                            )
                            nc.tensor.matmul(
                                ps[:, :, :TCc], w1[g][:, dc, us], xe[:, dc, :, :TCc],
                                start=False, stop=False,
                            )
                        for dc in range(DC):
                            nc.tensor.matmul(
                                ps[:, :, :TCc], w2[g][:, dc, us], xs[:, dc, :, :TCc],
                                start=False, stop=(dc == DC - 1),
                            )
                        on_act = (uh % 2) == 0
                        if use_bias:
                            if on_act:
                                nc.scalar.activation(
                                    pg[:, uh, :, :TCc], ps[:, :, :TCc], Act.Identity,
                                    bias=b_t[g][:, uh : uh + 1],
                                )
                            else:
                                nc.vector.tensor_scalar_add(
                                    pg[:, uh, :, :TCc], ps[:, :, :TCc],
                                    b_t[g][:, uh : uh + 1],
                                )
                        else:
                            if on_act:
                                nc.scalar.activation(
                                    pg[:, uh, :, :TCc], ps[:, :, :TCc], Act.Identity,
                                )
                            else:
                                nc.vector.tensor_copy(
                                    pg[:, uh, :, :TCc], ps[:, :, :TCc]
                                )

                pz, pr, ph = projs["z"], projs["r"], projs["h"]
                hch = [
                    hpool.tile([128, UH, BL // 2, TC], f32, tag=f"hch{gi}",
                               name=f"hch{gi}_{c}")
                    for gi in range(2)
                ]
                for trel in range(TCc):
                    for gi, (b0, b1) in enumerate(GROUPS):
                        if trel == 0 and c == 0:
                            h = prev_h[gi][:, :, :]
                        elif trel == 0:
                            h = prev_h[gi][:, :, :, prev_tc - 1]
                        else:
                            h = hch[gi][:, :, :, trel - 1]
                        xz_t = pz[:, :, b0:b1, trel]
                        xr_t = pr[:, :, b0:b1, trel]
                        xh_t = ph[:, :, b0:b1, trel]

                        t1in = spool.tile(
                            [128, UH, BL // 2], f32, tag=f"t1in{gi}", name=f"t1in{gi}"
                        )
                        zin = spool.tile(
                            [128, UH, BL // 2], f32, tag=f"zin{gi}", name=f"zin{gi}"
                        )
                        if use_memory:
                            hm_r = spool.tile(
                                [128, UH, BL // 2], f32, tag=f"hmr{gi}", name=f"hmr{gi}"
                            )
                            hm_z = spool.tile(
                                [128, UH, BL // 2], f32, tag=f"hmz{gi}", name=f"hmz{gi}"
                            )
                            nc.vector.tensor_mul(
                                hm_r[:, :, :], h, mr_t[:, :, b0:b1]
                            )
                            nc.vector.tensor_add(t1in[:, :, :], hm_r[:, :, :], xr_t)
                            nc.vector.tensor_mul(
                                hm_z[:, :, :], h, mz_t[:, :, b0:b1]
                            )
                            nc.vector.tensor_add(zin[:, :, :], hm_z[:, :, :], xz_t)
                        else:
                            nc.vector.tensor_add(t1in[:, :, :], h, xr_t)
                            nc.vector.tensor_add(zin[:, :, :], h, xz_t)

                        t1 = spool.tile(
                            [128, UH, BL // 2], f32, tag=f"t1{gi}", name=f"t1{gi}"
                        )
                        zp = spool.tile(
                            [128, UH, BL // 2], f32, tag=f"zp{gi}", name=f"zp{gi}"
                        )
                        nc.scalar.activation(t1[:, :, :], t1in[:, :, :], Act.Tanh)
                        # z' = sigmoid(-zin) = 1 - z
                        nc.scalar.activation(
                            zp[:, :, :], zin[:, :, :], Act.Sigmoid, scale=-1.0
                        )

                        # rh = (t1 + 1) * h = r * h
                        rh = spool.tile(
                            [128, UH, BL // 2], f32, tag=f"rh{gi}", name=f"rh{gi}"
                        )
                        nc.vector.scalar_tensor_tensor(
                            rh[:, :, :], t1[:, :, :], 1.0, h, Alu.add, Alu.mult
                        )
                        hhin = spool.tile(
                            [128, UH, BL // 2], f32, tag=f"hhin{gi}", name=f"hhin{gi}"
                        )
                        nc.vector.tensor_add(hhin[:, :, :], rh[:, :, :], xh_t)
                        hh = spool.tile(
                            [128, UH, BL // 2], f32, tag=f"hh{gi}", name=f"hh{gi}"
                        )
                        nc.scalar.activation(hh[:, :, :], hhin[:, :, :], Act.Tanh)

                        # h' = z'*hh + (1-z')*h  ==  e2 - a with
                        # e2 = z'*hh, a = (z'-1)*h
                        a = spool.tile(
                            [128, UH, BL // 2], f32, tag=f"a{gi}", name=f"a{gi}"
                        )
                        nc.vector.scalar_tensor_tensor(
                            a[:, :, :], zp[:, :, :], 1.0, h, Alu.subtract, Alu.mult
                        )
                        e2 = spool.tile(
                            [128, UH, BL // 2], f32, tag=f"e2{gi}", name=f"e2{gi}"
                        )
                        nc.vector.tensor_mul(e2[:, :, :], zp[:, :, :], hh[:, :, :])
                        nc.vector.tensor_sub(
                            hch[gi][:, :, :, trel], e2[:, :, :], a[:, :, :]
                        )

                for uh in range(UH):
                    for gi, (b0, b1) in enumerate(GROUPS):
                        nc.sync.dma_start(
                            outT_r[uh, :, b0:b1, t0 : t0 + TCc],
                            hch[gi][:, uh, :, :TCc],
                        )
                prev_h = hch
                prev_tc = TCc
                t0 += TCc

    nc.compile()
    return nc


def _get_nc(T_, TC, use_memory, use_bias):
    key = (T_, TC, use_memory, use_bias)
    if key not in _CACHE:
        _CACHE[key] = _build(T_, TC, use_memory, use_bias)
    return _CACHE[key]


def kernel(
    x,
    kernel_z,
    kernel_r,
    kernel_h,
    memory_z,
    memory_r,
    bias_z,
    bias_r,
    bias_h,
):
    from concourse import bass_utils

    x = np.asarray(x, dtype=np.float32)
    Ks = {
        "z": np.asarray(kernel_z, dtype=np.float32),
        "r": np.asarray(kernel_r, dtype=np.float32),
        "h": np.asarray(kernel_h, dtype=np.float32),
    }
    mem = {
        "z": np.asarray(memory_z, dtype=np.float32),
        "r": np.asarray(memory_r, dtype=np.float32),
    }
    bias = {
        "z": np.asarray(bias_z, dtype=np.float32),
        "r": np.asarray(bias_r, dtype=np.float32),
        "h": np.asarray(bias_h, dtype=np.float32),
    }

    B_, T_, D_ = x.shape
    assert (B_, D_) == (B, D), (x.shape,)
    TC = int(os.environ.get("BRU_TC", "32"))

    use_memory = not all(np.all(m == 1.0) for m in mem.values())
    use_bias = not all(np.all(b == 0.0) for b in bias.values())

    nc = _get_nc(T_, TC, use_memory, use_bias)

    # Split weights once (shared across cores).  The z-gate weights/bias are
    # pre-halved: the kernel computes tau = tanh(0.5*zin) instead of
    # sigmoid(zin).
    w1_full = {}
    w2_full = {}
    for g, K in Ks.items():
        if g == "z":
            K = K * np.float32(0.5)
        K1 = K.astype(np.float16)
        K2s = ((K - K1.astype(np.float32)) * 4096.0).astype(np.float16)
        w1_full[g] = K1
        w2_full[g] = K2s

    in_maps = []
    for c in range(NCORES):
        bg, ug = divmod(c, NUG)
        xc = x[bg * BL : (bg + 1) * BL].reshape(BL * T_, D)
        xcT = np.ascontiguousarray(xc.T)  # [D, NTOK] fp32
        A = xcT.astype(np.float16)
        e = (xcT - A.astype(np.float32)).astype(np.float16)
        As = (A.astype(np.float32) * (2.0 ** -12)).astype(np.float16)
        us = slice(ug * UHALF, (ug + 1) * UHALF)
        m = {"xA": A, "xE": e, "xS": As}
        for g in "zrh":
            m[f"k1{g}"] = np.ascontiguousarray(w1_full[g][:, us])
            m[f"k2{g}"] = np.ascontiguousarray(w2_full[g][:, us])
        if use_memory:
            # element (p, uh, b) = mem[ug*UHALF + uh*128 + p], pre-scaled
            for name, v, sc_ in (
                ("mzb", mem["z"], 0.25),
                ("mrb", mem["r"], 0.5),
            ):
                mv = (v[us] * np.float32(sc_)).reshape(UH, 128).T  # [128, UH]
                m[name] = np.ascontiguousarray(
                    np.broadcast_to(mv[:, :, None], (128, UH, BL))
                )
        if use_bias:
            for g in "zrh":
                bv = bias[g][us]
                if g == "z":
                    bv = bv * np.float32(0.5)
                m[f"bt{g}"] = np.ascontiguousarray(bv.reshape(UH, 128).T)
        in_maps.append(m)

    res = bass_utils.run_bass_kernel_spmd(nc, in_maps, core_ids=list(range(NCORES)))

    out = np.empty((B, T_, U), dtype=np.float32)
    for c in range(NCORES):
        bg, ug = divmod(c, NUG)
        oT = res.results[c]["outT"]  # [UHALF, BL*T_] holding v = 2h
        out[bg * BL : (bg + 1) * BL, :, ug * UHALF : (ug + 1) * UHALF] = (
            oT.reshape(UHALF, BL, T_).transpose(1, 2, 0)
        )
    out *= np.float32(0.5)
    return out


# revision 12
# speedup vs baseline: 47.7916x; 1.0040x over previous
"""BRU (bistable recurrent unit) cell kernel for 8 Trainium2 NeuronCores.

Hardcoded problem: B=64, T=512, D=1024, U=1024, fp32.

Sharding: 8 cores = 4 batch-groups (16 batches each) x 2 unit-groups
(512 units each).  Per core the three input projections
    projT[u, token] = K[d,u].T @ xT[d, token],   token = b*512 + t
run on the PE in fp16 with a 3-term split for fp32-grade accuracy:
    x @ K  ~=  A@K1 + e@K1 + (A*2^-12)@(K2*2^12)
with A = fp16(x), e = fp16(x - A), K1 = fp16(K), K2 = K - K1 (scaled by
2^12 into fp16 normal range; the power-of-two scales cancel exactly).
All terms run at 1 cycle/row on the PE and accumulate in fp32 PSUM.

The 512-step recurrence is elementwise with u on partitions:
state [128, (uh=4, b=16)].  Per step: 7 DVE ops + 3 ACT ops using
scalar_tensor_tensor fusions and h' = z'*hh + (1-z')*h with
z' = sigmoid(-zin) = 1-z.  Projections are chunked TC steps at a time
(double-buffered) so the PE runs ahead of the recurrence; PSUM->SBUF
copies ride on the Scalar engine and fold in the bias when nonzero.
"""

import os

import numpy as np

B, T, D, U = 64, 512, 1024, 1024
NCORES = 8
NBG = 4  # batch groups
NUG = 2  # unit groups
BL = B // NBG  # 16 batches per core
UHALF = U // NUG  # 512 units per core
UH = UHALF // 128  # 4 u-chunks

_CACHE: dict = {}


def _build(T_, TC, use_memory, use_bias):
    """Build and compile the per-core Bass program."""
    import concourse.mybir as mybir
    from concourse import bacc
    from concourse.tile import TileContext

    f32 = mybir.dt.float32
    f16 = mybir.dt.float16
    Alu = mybir.AluOpType
    Act = mybir.ActivationFunctionType

    NTOK = BL * T_
    NCH = T_ // TC
    DC = D // 128  # 8 d-chunks

    nc = bacc.Bacc("TRN2", target_bir_lowering=False, debug=False)

    xA = nc.dram_tensor("xA", [D, NTOK], f16, kind="ExternalInput").ap()
    xE = nc.dram_tensor("xE", [D, NTOK], f16, kind="ExternalInput").ap()
    xS = nc.dram_tensor("xS", [D, NTOK], f16, kind="ExternalInput").ap()
    k1 = {}
    k2 = {}
    for g in "zrh":
        k1[g] = nc.dram_tensor(f"k1{g}", [D, UHALF], f16, kind="ExternalInput").ap()
        k2[g] = nc.dram_tensor(f"k2{g}", [D, UHALF], f16, kind="ExternalInput").ap()
    if use_memory:
        mzb = nc.dram_tensor("mzb", [128, UH, BL], f32, kind="ExternalInput").ap()
        mrb = nc.dram_tensor("mrb", [128, UH, BL], f32, kind="ExternalInput").ap()
    if use_bias:
        bts = {
            g: nc.dram_tensor(f"bt{g}", [128, UH], f32, kind="ExternalInput").ap()
            for g in "zrh"
        }
    outT = nc.dram_tensor("outT", [UHALF, NTOK], f32, kind="ExternalOutput").ap()

    xA_r = xA.rearrange("(dc p) (b t) -> dc p b t", dc=DC, b=BL)
    xE_r = xE.rearrange("(dc p) (b t) -> dc p b t", dc=DC, b=BL)
    xS_r = xS.rearrange("(dc p) (b t) -> dc p b t", dc=DC, b=BL)
    outT_r = outT.rearrange("(uh p) (b t) -> uh p b t", uh=UH, b=BL)

    # Tapered chunk schedule: full TC chunks, then short final chunks so the
    # exposed scan tail after the last matmul is only a few steps long.
    chunks = []
    rem = T_
    taper = []
    full = (T_ - sum(taper)) // TC
    chunks = [TC] * full + taper
    assert sum(chunks) == T_, (chunks, T_)

    with TileContext(nc) as tc:
        with (
            tc.tile_pool(name="weights", bufs=1) as wpool,
            tc.tile_pool(name="xin", bufs=2) as xpool,
            tc.tile_pool(name="proj", bufs=2) as ppool,
            tc.tile_pool(name="hout", bufs=3) as hpool,
            tc.tile_pool(name="tmp", bufs=12) as spool,
            tc.tile_pool(name="misc", bufs=1) as mpool,
            tc.tile_pool(name="psum", bufs=8, space="PSUM") as qpool,
        ):
            # Startup order: z-gate weights, then the first x chunk, then
            # the remaining weights, so the PE's first PSUM group can start
            # as early as possible.
            TC0 = chunks[0]
            w1 = {}
            w2 = {}
            for g in "zrh":
                w1[g] = wpool.tile([128, DC, UHALF], f16, tag=f"w1{g}", name=f"w1{g}")
                w2[g] = wpool.tile([128, DC, UHALF], f16, tag=f"w2{g}", name=f"w2{g}")
            nc.sync.dma_start(
                w1["z"][:, :, :], k1["z"].rearrange("(dc p) u -> p dc u", p=128)
            )
            xa = xpool.tile([128, DC, BL, TC], f16, tag="xa", name="xa_0")
            xe = xpool.tile([128, DC, BL, TC], f16, tag="xe", name="xe_0")
            xs = xpool.tile([128, DC, BL, TC], f16, tag="xs", name="xs_0")
            for dc in range(DC):
                nc.sync.dma_start(xa[:, dc, :, :TC0], xA_r[dc, :, :, 0:TC0])
                nc.sync.dma_start(xe[:, dc, :, :TC0], xE_r[dc, :, :, 0:TC0])
            nc.sync.dma_start(
                w2["z"][:, :, :], k2["z"].rearrange("(dc p) u -> p dc u", p=128)
            )
            for dc in range(DC):
                nc.sync.dma_start(xs[:, dc, :, :TC0], xS_r[dc, :, :, 0:TC0])
            first_x = (xa, xe, xs)
            for g in "rh":
                nc.sync.dma_start(
                    w1[g][:, :, :], k1[g].rearrange("(dc p) u -> p dc u", p=128)
                )
                nc.sync.dma_start(
                    w2[g][:, :, :], k2[g].rearrange("(dc p) u -> p dc u", p=128)
                )
            if use_memory:
                # host passes mzb = 0.25*m_z, mrb = 0.5*m_r broadcasts
                mz4_t = mpool.tile([128, UH, BL], f32, tag="mz4", name="mz4")
                mr2_t = mpool.tile([128, UH, BL], f32, tag="mr2", name="mr2")
                nc.sync.dma_start(mz4_t[:, :, :], mzb[:, :, :])
                nc.sync.dma_start(mr2_t[:, :, :], mrb[:, :, :])
            if use_bias:
                b_t = {}
                for g in "zrh":
                    b_t[g] = mpool.tile([128, UH], f32, tag=f"b{g}", name=f"b{g}")
                    nc.sync.dma_start(b_t[g][:, :], bts[g][:, :])

            h0 = []
            for gi in range(2):
                h0g = mpool.tile([128, UH, BL // 2], f32, tag=f"h0{gi}", name=f"h0{gi}")
                nc.gpsimd.memset(h0g[:, :, :], 0.0)
                h0.append(h0g)

            # batch halves for the two-group pipelined scan
            GROUPS = ((0, BL // 2), (BL // 2, BL))

            prev_h = h0
            prev_tc = None
            t0 = 0
            for c, TCc in enumerate(chunks):
                if c == 0:
                    xa, xe, xs = first_x
                else:
                    xa = xpool.tile([128, DC, BL, TC], f16, tag="xa", name=f"xa_{c}")
                    xe = xpool.tile([128, DC, BL, TC], f16, tag="xe", name=f"xe_{c}")
                    xs = xpool.tile([128, DC, BL, TC], f16, tag="xs", name=f"xs_{c}")
                    for dc in range(DC):
                        nc.sync.dma_start(
                            xa[:, dc, :, :TCc], xA_r[dc, :, :, t0 : t0 + TCc]
                        )
                        nc.sync.dma_start(
                            xe[:, dc, :, :TCc], xE_r[dc, :, :, t0 : t0 + TCc]
                        )
                        nc.sync.dma_start(
                            xs[:, dc, :, :TCc], xS_r[dc, :, :, t0 : t0 + TCc]
                        )
                projs = {}
                for g in "zrh":
                    pg = ppool.tile(
                        [128, UH, BL, TC], f32, tag=f"p{g}", name=f"p{g}_{c}"
                    )
                    projs[g] = pg
                    for uh in range(UH):
                        us = slice(uh * 128, (uh + 1) * 128)
                        ps = qpool.tile([128, BL, TC], f32, tag="ps")
                        for dc in range(DC):
                            nc.tensor.matmul(
                                ps[:, :, :TCc], w1[g][:, dc, us], xa[:, dc, :, :TCc],
                                start=(dc == 0), stop=False,
                            )
                            nc.tensor.matmul(
                                ps[:, :, :TCc], w1[g][:, dc, us], xe[:, dc, :, :TCc],
                                start=False, stop=False,
                            )
                        for dc in range(DC):
                            nc.tensor.matmul(
                                ps[:, :, :TCc], w2[g][:, dc, us], xs[:, dc, :, :TCc],
                                start=False, stop=(dc == DC - 1),
                            )
                        on_act = (uh % 2) == 0
                        if use_bias:
                            if on_act:
                                nc.scalar.activation(
                                    pg[:, uh, :, :TCc], ps[:, :, :TCc], Act.Identity,
                                    bias=b_t[g][:, uh : uh + 1],
                                )
                            else:
                                nc.vector.tensor_scalar_add(
                                    pg[:, uh, :, :TCc], ps[:, :, :TCc],
                                    b_t[g][:, uh : uh + 1],
                                )
                        else:
                            if on_act:
                                nc.scalar.activation(
                                    pg[:, uh, :, :TCc], ps[:, :, :TCc], Act.Identity,
                                )
                            else:
                                nc.vector.tensor_copy(
                                    pg[:, uh, :, :TCc], ps[:, :, :TCc]
                                )

                pz, pr, ph = projs["z"], projs["r"], projs["h"]
                hch = [
                    hpool.tile([128, UH, BL // 2, TC], f32, tag=f"hch{gi}",
                               name=f"hch{gi}_{c}")
                    for gi in range(2)
                ]
                for trel in range(TCc):
                    for gi, (b0, b1) in enumerate(GROUPS):
                        if trel == 0 and c == 0:
                            h = prev_h[gi][:, :, :]
                        elif trel == 0:
                            h = prev_h[gi][:, :, :, prev_tc - 1]
                        else:
                            h = hch[gi][:, :, :, trel - 1]
                        xz_t = pz[:, :, b0:b1, trel]
                        xr_t = pr[:, :, b0:b1, trel]
                        xh_t = ph[:, :, b0:b1, trel]

                        t1in = spool.tile(
                            [128, UH, BL // 2], f32, tag=f"t1in{gi}", name=f"t1in{gi}"
                        )
                        zin = spool.tile(
                            [128, UH, BL // 2], f32, tag=f"zin{gi}", name=f"zin{gi}"
                        )
                        if use_memory:
                            hm_r = spool.tile(
                                [128, UH, BL // 2], f32, tag=f"hmr{gi}", name=f"hmr{gi}"
                            )
                            hm_z = spool.tile(
                                [128, UH, BL // 2], f32, tag=f"hmz{gi}", name=f"hmz{gi}"
                            )
                            nc.vector.tensor_mul(
                                hm_r[:, :, :], h, mr_t[:, :, b0:b1]
                            )
                            nc.vector.tensor_add(t1in[:, :, :], hm_r[:, :, :], xr_t)
                            nc.vector.tensor_mul(
                                hm_z[:, :, :], h, mz_t[:, :, b0:b1]
                            )
                            nc.vector.tensor_add(zin[:, :, :], hm_z[:, :, :], xz_t)
                        else:
                            nc.vector.tensor_add(t1in[:, :, :], h, xr_t)
                            nc.vector.tensor_add(zin[:, :, :], h, xz_t)

                        t1 = spool.tile(
                            [128, UH, BL // 2], f32, tag=f"t1{gi}", name=f"t1{gi}"
                        )
                        zp = spool.tile(
                            [128, UH, BL // 2], f32, tag=f"zp{gi}", name=f"zp{gi}"
                        )
                        nc.scalar.activation(t1[:, :, :], t1in[:, :, :], Act.Tanh)
                        # z' = sigmoid(-zin) = 1 - z
                        nc.scalar.activation(
                            zp[:, :, :], zin[:, :, :], Act.Sigmoid, scale=-1.0
                        )

                        # rh = (t1 + 1) * h = r * h
                        rh = spool.tile(
                            [128, UH, BL // 2], f32, tag=f"rh{gi}", name=f"rh{gi}"
                        )
                        nc.vector.scalar_tensor_tensor(
                            rh[:, :, :], t1[:, :, :], 1.0, h, Alu.add, Alu.mult
                        )
                        hhin = spool.tile(
                            [128, UH, BL // 2], f32, tag=f"hhin{gi}", name=f"hhin{gi}"
                        )
                        nc.vector.tensor_add(hhin[:, :, :], rh[:, :, :], xh_t)
                        hh = spool.tile(
                            [128, UH, BL // 2], f32, tag=f"hh{gi}", name=f"hh{gi}"
                        )
                        nc.scalar.activation(hh[:, :, :], hhin[:, :, :], Act.Tanh)

                        # h' = z'*hh + (1-z')*h  ==  e2 - a with
                        # e2 = z'*hh, a = (z'-1)*h
                        a = spool.tile(
                            [128, UH, BL // 2], f32, tag=f"a{gi}", name=f"a{gi}"
                        )
                        nc.vector.scalar_tensor_tensor(
                            a[:, :, :], zp[:, :, :], 1.0, h, Alu.subtract, Alu.mult
                        )
                        e2 = spool.tile(
                            [128, UH, BL // 2], f32, tag=f"e2{gi}", name=f"e2{gi}"
                        )
                        nc.vector.tensor_mul(e2[:, :, :], zp[:, :, :], hh[:, :, :])
                        nc.vector.tensor_sub(
                            hch[gi][:, :, :, trel], e2[:, :, :], a[:, :, :]
                        )

                for uh in range(UH):
                    for gi, (b0, b1) in enumerate(GROUPS):
                        nc.sync.dma_start(
                            outT_r[uh, :, b0:b1, t0 : t0 + TCc],
                            hch[gi][:, uh, :, :TCc],
                        )
                prev_h = hch
                prev_tc = TCc
                t0 += TCc

    nc.compile()
    return nc


def _get_nc(T_, TC, use_memory, use_bias):
    key = (T_, TC, use_memory, use_bias)
    if key not in _CACHE:
        _CACHE[key] = _build(T_, TC, use_memory, use_bias)
    return _CACHE[key]


def kernel(
    x,
    kernel_z,
    kernel_r,
    kernel_h,
    memory_z,
    memory_r,
    bias_z,
    bias_r,
    bias_h,
):
    from concourse import bass_utils

    x = np.asarray(x, dtype=np.float32)
    Ks = {
        "z": np.asarray(kernel_z, dtype=np.float32),
        "r": np.asarray(kernel_r, dtype=np.float32),
        "h": np.asarray(kernel_h, dtype=np.float32),
    }
    mem = {
        "z": np.asarray(memory_z, dtype=np.float32),
        "r": np.asarray(memory_r, dtype=np.float32),
    }
    bias = {
        "z": np.asarray(bias_z, dtype=np.float32),
        "r": np.asarray(bias_r, dtype=np.float32),
        "h": np.asarray(bias_h, dtype=np.float32),
    }

    B_, T_, D_ = x.shape
    assert (B_, D_) == (B, D), (x.shape,)
    TC = int(os.environ.get("BRU_TC", "32"))

    use_memory = not all(np.all(m == 1.0) for m in mem.values())
    use_bias = not all(np.all(b == 0.0) for b in bias.values())

    nc = _get_nc(T_, TC, use_memory, use_bias)

    # Split weights once (shared across cores).  The z-gate weights/bias are
    # pre-halved: the kernel computes tau = tanh(0.5*zin) instead of
    # sigmoid(zin).
    w1_full = {}
    w2_full = {}
    for g, K in Ks.items():
        if g == "z":
            K = K * np.float32(0.5)
        K1 = K.astype(np.float16)
        K2s = ((K - K1.astype(np.float32)) * 4096.0).astype(np.float16)
        w1_full[g] = K1
        w2_full[g] = K2s

    in_maps = []
    for c in range(NCORES):
        bg, ug = divmod(c, NUG)
        xc = x[bg * BL : (bg + 1) * BL].reshape(BL * T_, D)
        xcT = np.ascontiguousarray(xc.T)  # [D, NTOK] fp32
        A = xcT.astype(np.float16)
        e = (xcT - A.astype(np.float32)).astype(np.float16)
        As = (A.astype(np.float32) * (2.0 ** -12)).astype(np.float16)
        us = slice(ug * UHALF, (ug + 1) * UHALF)
        m = {"xA": A, "xE": e, "xS": As}
        for g in "zrh":
            m[f"k1{g}"] = np.ascontiguousarray(w1_full[g][:, us])
            m[f"k2{g}"] = np.ascontiguousarray(w2_full[g][:, us])
        if use_memory:
            # element (p, uh, b) = mem[ug*UHALF + uh*128 + p], pre-scaled
            for name, v, sc_ in (
                ("mzb", mem["z"], 0.25),
                ("mrb", mem["r"], 0.5),
            ):
                mv = (v[us] * np.float32(sc_)).reshape(UH, 128).T  # [128, UH]
                m[name] = np.ascontiguousarray(
                    np.broadcast_to(mv[:, :, None], (128, UH, BL))
                )
        if use_bias:
            for g in "zrh":
                bv = bias[g][us]
                if g == "z":
                    bv = bv * np.float32(0.5)
                m[f"bt{g}"] = np.ascontiguousarray(bv.reshape(UH, 128).T)
        in_maps.append(m)

    res = bass_utils.run_bass_kernel_spmd(nc, in_maps, core_ids=list(range(NCORES)))

    out = np.empty((B, T_, U), dtype=np.float32)
    for c in range(NCORES):
        bg, ug = divmod(c, NUG)
        oT = res.results[c]["outT"]  # [UHALF, BL*T_] holding v = 2h
        out[bg * BL : (bg + 1) * BL, :, ug * UHALF : (ug + 1) * UHALF] = (
            oT.reshape(UHALF, BL, T_).transpose(1, 2, 0)
        )
    out *= np.float32(0.5)
    return out
